# revision 2
# baseline (speedup 1.0000x reference)
"""Trainium2 Bass kernel for nn_MultiHeadAttention_58531814310501.

Full-input contract: kernel(**inputs) takes the unsharded tensors
(x [2,2048,2048], wq/wk/wv/wo [2048,2048], bq/bk/bv/bo [2048]) and
returns the full [2,2048,2048] output.

Sharding (8 NeuronCores): tensor-parallel over heads x data-parallel
over batch. Core c handles batch b=c//4 and heads 4*(c%4)..4*(c%4)+3
(a 512-wide feature slice). Each core computes its 4 heads' attention
and a row-sharded partial of the out-projection for its whole batch
element; the host sums the 4 partials per batch and adds bo.

On-core dataflow (all matmuls in float32r = full-rate fp32):
  phase 1: Q^T,K^T = (wq/wk slice)^T-stationary @ x^T-moving  [e,t] layout
  phase 2: V = x^T-chunks-stationary @ wv^T-moving            [t,e] layout
  phase 3: per (head, 512-token q-block):
           scores^T[k,q] = K^T-станционary @ Q^T-moving  (scale folded
           into wq host-side), exp on ACT (no max-sub: scores are
           O(1) by construction), denominator = DVE chunk-sum +
           PE ones-reduce, out_h^T = V-stationary @ P^T-moving,
           normalize by PE-broadcast reciprocal.
  phase 4: out[t,e] partial = attnOut^T-chunks-stationary @ wo^T-moving.
"""
import sys

if "/opt/trn_rl_repo" not in sys.path:
    sys.path.insert(0, "/opt/trn_rl_repo")

import math
import numpy as np

B = 2
S = 2048
D = 2048
P = 128
NH = 16
DH = 128
FPC = 512          # features (head-dims) per core
EC = FPC // P      # 4 feature chunks per core
DC = D // P        # 16 contraction chunks
TC = S // 512      # 4 token chunks of 512
QB = 512           # attention q-block width
NQB = S // QB
HPC = 4            # heads per core
N_CORES = 8

_STATE = {}


def _build_nc():
    import concourse.bass as bass
    import concourse.mybir as mybir
    import concourse.tile as tile
    from concourse import bacc

    F32 = mybir.dt.float32
    F32R = mybir.dt.float32r
    AF = mybir.ActivationFunctionType
    ADD = mybir.AluOpType.add
    MULT = mybir.AluOpType.mult
    ts = bass.ts

    nc = bacc.Bacc(trn_type="TRN2")
    xT = nc.declare_dram_parameter("xT", [D, S], F32R, isOutput=False)
    wqT = nc.declare_dram_parameter("wqT", [D, FPC], F32R, isOutput=False)
    wkT = nc.declare_dram_parameter("wkT", [D, FPC], F32R, isOutput=False)
    wvT = nc.declare_dram_parameter("wvT", [D, FPC], F32R, isOutput=False)
    woT = nc.declare_dram_parameter("woT", [FPC, D], F32R, isOutput=False)
    bqc = nc.declare_dram_parameter("bqc", [EC, P], F32, isOutput=False)
    bkc = nc.declare_dram_parameter("bkc", [EC, P], F32, isOutput=False)
    bvr = nc.declare_dram_parameter("bvr", [1, FPC], F32R, isOutput=False)
    ones_bc = nc.declare_dram_parameter("ones_bc", [1, P], F32R, isOutput=False)
    ones_rd = nc.declare_dram_parameter("ones_rd", [P, 1], F32R, isOutput=False)
    out = nc.declare_dram_parameter("out", [S, D], F32, isOutput=True)

    with tile.TileContext(nc) as tc:
        with tc.tile_pool(name="const", bufs=1) as cpool, \
             tc.tile_pool(name="persist", bufs=1) as ppool:
            ones_bc_t = cpool.tile([1, P], F32R, tag="onbc", name="ones_bc_t")
            nc.sync.dma_start(ones_bc_t[:], ones_bc[:, :])
            ones_rd_t = cpool.tile([P, 1], F32R, tag="onrd", name="ones_rd_t")
            nc.sync.dma_start(ones_rd_t[:], ones_rd[:, :])
            bq_t = []
            bk_t = []
            for e in range(EC):
                t = cpool.tile([P, 1], F32, tag=f"bq{e}", name=f"bq_t{e}")
                nc.sync.dma_start(t[:], bqc.ap()[e].rearrange("(p o) -> p o", o=1))
                bq_t.append(t)
                t = cpool.tile([P, 1], F32, tag=f"bk{e}", name=f"bk_t{e}")
                nc.sync.dma_start(t[:], bkc.ap()[e].rearrange("(p o) -> p o", o=1))
                bk_t.append(t)
            bv_t = cpool.tile([1, FPC], F32R, tag="bv", name="bv_t")
            nc.sync.dma_start(bv_t[:], bvr[:, :])

            QT = [ppool.tile([P, S], F32R, tag=f"QT{e}", name=f"QT{e}")
                  for e in range(EC)]
            KT = [ppool.tile([P, S], F32R, tag=f"KT{e}", name=f"KT{e}")
                  for e in range(EC)]
            V = [ppool.tile([P, FPC], F32R, tag=f"V{t}", name=f"V{t}")
                 for t in range(DC)]
            AO = [ppool.tile([P, S], F32R, tag=f"AO{h}", name=f"AO{h}")
                  for h in range(HPC)]

            # ---------------- phase 1: Q^T, K^T projections ----------------
            with tc.tile_pool(name="w1", bufs=1) as wpool, \
                 tc.tile_pool(name="x1", bufs=4) as xpool, \
                 tc.tile_pool(name="ps1", bufs=1, space="PSUM") as psp:
                wq_t = []
                wk_t = []
                for d in range(DC):
                    t = wpool.tile([P, FPC], F32R, tag=f"wq{d}", name=f"wq_t{d}")
                    nc.sync.dma_start(t[:], wqT.ap()[ts(d, P), :])
                    wq_t.append(t)
                    t = wpool.tile([P, FPC], F32R, tag=f"wk{d}", name=f"wk_t{d}")
                    nc.sync.dma_start(t[:], wkT.ap()[ts(d, P), :])
                    wk_t.append(t)
                for tcx in range(TC):
                    pq = [psp.tile([P, 512], F32, tag=f"pq{e}", name=f"pq{e}")
                          for e in range(EC)]
                    pk = [psp.tile([P, 512], F32, tag=f"pk{e}", name=f"pk{e}")
                          for e in range(EC)]
                    for d in range(DC):
                        xt = xpool.tile([P, 512], F32R, tag="xt", name="xt")
                        nc.sync.dma_start(xt[:], xT.ap()[ts(d, P), ts(tcx, 512)])
                        for e in range(EC):
                            nc.tensor.matmul(pq[e][:], wq_t[d][:, ts(e, P)], xt[:],
                                             start=(d == 0), stop=(d == DC - 1))
                        for e in range(EC):
                            nc.tensor.matmul(pk[e][:], wk_t[d][:, ts(e, P)], xt[:],
                                             start=(d == 0), stop=(d == DC - 1))
                    for e in range(EC):
                        nc.vector.tensor_scalar(QT[e][:, ts(tcx, 512)], pq[e][:],
                                                bq_t[e][:], scalar2=None, op0=ADD)
                        nc.vector.tensor_scalar(KT[e][:, ts(tcx, 512)], pk[e][:],
                                                bk_t[e][:], scalar2=None, op0=ADD)

            # ---------------- phase 2: V projection ----------------
            with tc.tile_pool(name="w2", bufs=1) as wpool2, \
                 tc.tile_pool(name="x2", bufs=4) as xpool2, \
                 tc.tile_pool(name="ps2", bufs=2, space="PSUM") as psp2:
                wv_t = []
                for d in range(DC):
                    t = wpool2.tile([P, FPC], F32R, tag=f"wv{d}", name=f"wv_t{d}")
                    nc.sync.dma_start(t[:], wvT.ap()[ts(d, P), :])
                    wv_t.append(t)
                for tcx in range(TC):
                    pv = [psp2.tile([P, FPC], F32, tag=f"pv{t}", name=f"pv{t}")
                          for t in range(4)]
                    for d in range(DC):
                        xt2 = xpool2.tile([P, 512], F32R, tag="xt2", name="xt2")
                        nc.sync.dma_start(xt2[:], xT.ap()[ts(d, P), ts(tcx, 512)])
                        for t in range(4):
                            nc.tensor.matmul(pv[t][:], xt2[:, ts(t, P)], wv_t[d][:],
                                             start=(d == 0), stop=False)
                    for t in range(4):
                        nc.tensor.matmul(pv[t][:], ones_bc_t[:], bv_t[:],
                                         start=False, stop=True)
                        nc.vector.tensor_copy(V[tcx * 4 + t][:], pv[t][:])

            # ---------------- phase 3: attention ----------------
            with tc.tile_pool(name="pt3", bufs=2) as ptpool, \
                 tc.tile_pool(name="t3", bufs=1) as tpool, \
                 tc.tile_pool(name="ps3", bufs=1, space="PSUM") as psp3:
                for h in range(HPC):
                    for qb in range(NQB):
                        qsl = ts(qb, QB)
                        pt = ptpool.tile([P, DC, QB], F32R, tag="pt", name="pt")
                        acc = tpool.tile([P, QB], F32R, tag="acc", name="acc")
                        for kc in range(DC):
                            ps = psp3.tile([P, QB], F32, tag="ps", name="ps",
                                           bufs=3)
                            nc.tensor.matmul(ps[:], KT[h][:, ts(kc, P)],
                                             QT[h][:, qsl], start=True, stop=True)
                            nc.scalar.activation(pt[:, kc, :], ps[:], AF.Exp)
                            if kc == 0:
                                nc.vector.tensor_copy(acc[:], pt[:, 0, :])
                            else:
                                nc.vector.tensor_add(acc[:], acc[:], pt[:, kc, :])
                        pav = psp3.tile([P, QB], F32, tag="pav", name="pav",
                                        bufs=2)
                        for kc in range(DC):
                            nc.tensor.matmul(pav[:], V[kc][:, ts(h, P)],
                                             pt[:, kc, :],
                                             start=(kc == 0), stop=(kc == DC - 1))
                        pd = psp3.tile([1, QB], F32, tag="pd", name="pd", bufs=1)
                        nc.tensor.matmul(pd[:], ones_rd_t[:], acc[:],
                                         start=True, stop=True)
                        rec = tpool.tile([1, QB], F32, tag="rec", name="rec")
                        nc.vector.reciprocal(rec[:], pd[:])
                        rec_r = tpool.tile([1, QB], F32R, tag="rec_r", name="rec_r")
                        nc.vector.tensor_copy(rec_r[:], rec[:])
                        pb = psp3.tile([P, QB], F32, tag="pb", name="pb", bufs=1)
                        nc.tensor.matmul(pb[:], ones_bc_t[:], rec_r[:],
                                         start=True, stop=True)
                        rb = tpool.tile([P, QB], F32, tag="rb", name="rb")
                        nc.scalar.copy(rb[:], pb[:])
                        nc.vector.tensor_tensor(AO[h][:, qsl], pav[:], rb[:], MULT)

            # ---------------- phase 4: out projection (partial) ----------------
            with tc.tile_pool(name="wo4", bufs=1) as wopool, \
                 tc.tile_pool(name="o4", bufs=4) as opool, \
                 tc.tile_pool(name="ps4", bufs=4, space="PSUM") as psp4:
                wo_t = [[None] * EC for _ in range(HPC)]
                for e in range(EC):
                    for k in range(HPC):
                        t = wopool.tile([P, 512], F32R, tag=f"wo{k}_{e}",
                                        name=f"wo_t{k}_{e}")
                        nc.sync.dma_start(t[:], woT.ap()[ts(k, P), ts(e, 512)])
                        wo_t[k][e] = t
                for e in range(EC):
                    for t in range(DC):
                        po = psp4.tile([P, 512], F32, tag="po", name="po")
                        for k in range(HPC):
                            nc.tensor.matmul(po[:], AO[k][:, ts(t, P)],
                                             wo_t[k][e][:],
                                             start=(k == 0), stop=(k == HPC - 1))
                        ot = opool.tile([P, 512], F32, tag="ot", name="ot")
                        nc.scalar.copy(ot[:], po[:])
                        nc.sync.dma_start(out.ap()[ts(t, P), ts(e, 512)], ot[:])

    nc.compile()
    return nc


def _get_exec():
    """Build (once) and cache the program + a reusable jitted SPMD executor."""
    if "exec" in _STATE:
        return _STATE["exec"]

    import jax
    import concourse.mybir as mybir
    from concourse import bass2jax
    from jax.sharding import Mesh, PartitionSpec
    from jax.experimental.shard_map import shard_map

    nc = _build_nc()
    bass2jax.install_neuronx_cc_hook()

    partition_name = (nc.partition_id_tensor.name
                      if nc.partition_id_tensor else None)
    in_names = []
    out_names = []
    out_avals = []
    for alloc in nc.m.functions[0].allocations:
        if not isinstance(alloc, mybir.MemoryLocationSet):
            continue
        name = alloc.memorylocations[0].name
        if alloc.kind == "ExternalInput":
            if name != partition_name:
                in_names.append(name)
        elif alloc.kind == "ExternalOutput":
            out_names.append(name)
            out_avals.append(jax.core.ShapedArray(
                tuple(alloc.tensor_shape), mybir.dt.np(alloc.dtype)))
    n_params = len(in_names)
    bind_names = list(in_names) + list(out_names)
    if partition_name is not None:
        bind_names.append(partition_name)

    def _body(*args):
        operands = list(args)
        if partition_name is not None:
            operands.append(bass2jax.partition_id_tensor())
        outs = bass2jax._bass_exec_p.bind(
            *operands,
            out_avals=tuple(out_avals),
            in_names=tuple(bind_names),
            out_names=tuple(out_names),
            lowering_input_output_aliases=(),
            sim_require_finite=True,
            sim_require_nnan=True,
            nc=nc,
        )
        return tuple(outs)

    devices = jax.devices()[:N_CORES]
    mesh = Mesh(np.asarray(devices), ("core",))
    n_outs = len(out_names)
    in_specs = (PartitionSpec("core"),) * (n_params + n_outs)
    out_specs = (PartitionSpec("core"),) * n_outs
    sharded = jax.jit(
        shard_map(_body, mesh=mesh, in_specs=in_specs, out_specs=out_specs,
                  check_rep=False),
        keep_unused=True,
    )
    # reusable (non-donated) zero buffers for the output-donation slots
    zeros = [
        jax.device_put(
            np.zeros((N_CORES * av.shape[0],) + tuple(av.shape[1:]), av.dtype),
            jax.sharding.NamedSharding(mesh, PartitionSpec("core")))
        for av in out_avals
    ]
    _STATE["exec"] = (sharded, in_names, out_names, out_avals, zeros, mesh)
    return _STATE["exec"]


def _make_in_maps(x, wq, bq, wk, bk, wv, bv, wo, bo):
    isq = np.float32(1.0 / math.sqrt(DH))
    ones_bc = np.ones((1, P), np.float32)
    ones_rd = np.ones((P, 1), np.float32)
    in_maps = []
    for c in range(N_CORES):
        b, g = divmod(c, 4)
        fs = slice(FPC * g, FPC * (g + 1))
        in_maps.append({
            "xT": np.ascontiguousarray(x[b].T),
            "wqT": np.ascontiguousarray((wq[fs] * isq).T),
            "wkT": np.ascontiguousarray(wk[fs].T),
            "wvT": np.ascontiguousarray(wv[fs].T),
            "woT": np.ascontiguousarray(wo[:, fs].T),
            "bqc": (bq[fs] * isq).reshape(EC, P),
            "bkc": bk[fs].reshape(EC, P).copy(),
            "bvr": bv[fs].reshape(1, FPC).copy(),
            "ones_bc": ones_bc,
            "ones_rd": ones_rd,
        })
    return in_maps


def _run(in_maps):
    import jax
    sharded, in_names, out_names, out_avals, zeros, mesh = _get_exec()
    concat_in = [
        np.concatenate([np.asarray(in_maps[c][name]) for c in range(N_CORES)],
                       axis=0)
        for name in in_names
    ]
    out_arrs = sharded(*concat_in, *zeros)
    res = []
    for c in range(N_CORES):
        res.append({
            name: np.asarray(out_arrs[i]).reshape(
                N_CORES, *out_avals[i].shape)[c]
            for i, name in enumerate(out_names)
        })
    return res


def kernel(x, wq, bq, wk, bk, wv, bv, wo, bo):
    x = np.asarray(x, np.float32)
    wq = np.asarray(wq, np.float32)
    bq = np.asarray(bq, np.float32)
    wk = np.asarray(wk, np.float32)
    bk = np.asarray(bk, np.float32)
    wv = np.asarray(wv, np.float32)
    bv = np.asarray(bv, np.float32)
    wo = np.asarray(wo, np.float32)
    bo = np.asarray(bo, np.float32)

    in_maps = _make_in_maps(x, wq, bq, wk, bk, wv, bv, wo, bo)
    res = _run(in_maps)

    full = np.empty((B, S, D), np.float32)
    for b in range(B):
        acc = res[4 * b]["out"].copy()
        for g in range(1, 4):
            acc += res[4 * b + g]["out"]
        full[b] = acc + bo[None, :]
    return full
